# revision 1
# baseline (speedup 1.0000x reference)
"""Trainium2 Bass kernel for the longtail Plackett-Luce loss.

Math (per batch row b):
    sum_exp  = sum_v exp(output[b, v])
    log_pl   = output[b, target[b]] - log(sum_exp)
    exp_s[k] = mask[k] * exp(output[b, longtail[b, k]])     mask = longtail > 0
    arg[k]   = (sum_exp - exp(output[b, target[b]])) - sum_{j<k} exp_s[j]
             ( == rev_cumsum(exp_s)[k] + other   in the reference formulation )
    tail     = sum_k mask[k] * (scores[k] - log(arg[k]))
    neg_like = -(log_pl + tail) + loss_weight[target[b]]

Sharding: batch rows split across 8 NeuronCores (512 rows each), loss_weight
replicated.  Per core the 512x32000 f32 slice (65.5 MB) is streamed through
SBUF in [128, 4000] tiles; the scalar engine does exp with fused row-sum
(accum_out).  The 51 per-row gathers (50 longtail + target) are one
indirect-DMA gather per 128-row group; loss_weight[target] is a second tiny
indirect gather.  The reverse cumsum uses tensor_tensor_scan.
"""

import sys

import numpy as np

sys.path.insert(0, "/opt/trn_rl_repo")

import concourse.bass as bass  # noqa: E402
import concourse.bacc as bacc  # noqa: E402
import concourse.tile as tile  # noqa: E402
from concourse import mybir  # noqa: E402
from concourse.bass_utils import run_bass_kernel_spmd  # noqa: E402

B, V, L = 4096, 32000, 50
NCORES = 8
RPC = B // NCORES  # 512 rows per core
P = 128            # SBUF partitions
G = RPC // P       # 4 row-groups per core
NCH = 8            # column chunks per row-group
C = V // NCH       # 4000 columns per chunk

F32 = mybir.dt.float32
I32 = mybir.dt.int32
ALU = mybir.AluOpType
ACTF = mybir.ActivationFunctionType

# Knobs test.py can flip for profiling.
TRACE = False
TRACE_KWARGS = {}
LAST_RESULTS = None

_NC_CACHE = None


def build_nc():
    nc = bacc.Bacc()
    out_t = nc.dram_tensor("output", [RPC, V], F32, kind="ExternalInput")
    tgt_t = nc.dram_tensor("tgt", [RPC, 1], I32, kind="ExternalInput")
    lt_t = nc.dram_tensor("lt", [RPC, L], I32, kind="ExternalInput")
    lw_t = nc.dram_tensor("lw", [1, V], F32, kind="ExternalInput")
    res_t = nc.dram_tensor("neg_like", [RPC, 1], F32, kind="ExternalOutput")

    out_ap = out_t[:, :]

    with tile.TileContext(nc) as tc:
        with (
            tc.tile_pool(name="stream", bufs=4) as stream,
            tc.tile_pool(name="scratch", bufs=2) as scratch,
            tc.tile_pool(name="small", bufs=4) as small,
            tc.tile_pool(name="consts", bufs=1) as consts,
        ):
            # rowoff[p] = p * V  (partition -> flat row offset within a group)
            rowoff = consts.tile([P, 1], I32)
            nc.gpsimd.iota(rowoff[:], pattern=[[0, 1]], base=0, channel_multiplier=V)
            neg1 = consts.tile([P, L], F32)
            nc.vector.memset(neg1[:], -1.0)

            for g in range(G):
                r0 = g * P

                # --- index prep + gathers ---
                lt_sb = small.tile([P, L], I32, tag="lt")
                nc.sync.dma_start(out=lt_sb[:], in_=lt_t[r0 : r0 + P, :])
                tgt_sb = small.tile([P, 1], I32, tag="tgt")
                nc.sync.dma_start(out=tgt_sb[:], in_=tgt_t[r0 : r0 + P, :])

                mask = small.tile([P, L], F32, tag="mask")
                nc.vector.tensor_scalar(
                    out=mask[:], in0=lt_sb[:], scalar1=0, scalar2=None,
                    op0=ALU.is_gt,
                )
                # clip(longtail, 0, V-1), matching the reference
                idx_clip = small.tile([P, L], I32, tag="idxclip")
                nc.gpsimd.tensor_scalar(
                    out=idx_clip[:], in0=lt_sb[:], scalar1=0, scalar2=V - 1,
                    op0=ALU.max, op1=ALU.min,
                )
                idx_all = small.tile([P, L + 1], I32, tag="idx")
                nc.gpsimd.tensor_tensor(
                    out=idx_all[:, 0:L], in0=idx_clip[:],
                    in1=rowoff[:].to_broadcast([P, L]), op=ALU.add,
                )
                nc.gpsimd.tensor_tensor(
                    out=idx_all[:, L : L + 1], in0=tgt_sb[:],
                    in1=rowoff[:], op=ALU.add,
                )

                # scores[p, k] = output[r0 + p, idx[p, k]]; col L = target score
                # HW indirect DMA consumes ONE index per dest partition-row,
                # so issue one [128,1] gather per tail position.
                scores = small.tile([P, L + 1], F32, tag="scores")
                for k in range(L + 1):
                    nc.gpsimd.indirect_dma_start(
                        out=scores[:, k : k + 1], out_offset=None,
                        in_=out_ap,
                        in_offset=bass.IndirectOffsetOnAxis(
                            ap=idx_all[:, k : k + 1], axis=1
                        ),
                        element_offset=r0 * V,
                    )
                curw = small.tile([P, 1], F32, tag="curw")
                nc.gpsimd.indirect_dma_start(
                    out=curw[:], out_offset=None,
                    in_=lw_t[:, :],
                    in_offset=bass.IndirectOffsetOnAxis(ap=tgt_sb[:], axis=1),
                )
                # DVE-side copies: later DVE consumers then have same-engine
                # deps (the DVE TensorTensor struct allows only ONE sync wait)
                scoresd = small.tile([P, L + 1], F32, tag="scoresd")
                nc.vector.tensor_copy(out=scoresd[:], in_=scores[:])

                # --- stream the row-group, exp + accumulate row sums ---
                acc = small.tile([P, NCH], F32, tag="acc")
                for ch in range(NCH):
                    x = stream.tile([P, C], F32, tag="x")
                    nc.sync.dma_start(
                        out=x[:], in_=out_ap[r0 : r0 + P, ch * C : (ch + 1) * C]
                    )
                    e = scratch.tile([P, C], F32, tag="e")
                    nc.scalar.activation(
                        out=e[:], in_=x[:], func=ACTF.Exp,
                        accum_out=acc[:, ch : ch + 1],
                    )
                sumexp = small.tile([P, 1], F32, tag="sumexp")
                nc.vector.tensor_reduce(
                    out=sumexp[:], in_=acc[:], axis=mybir.AxisListType.X, op=ALU.add
                )

                # --- tail term ---
                expt = small.tile([P, 1], F32, tag="expt")
                nc.scalar.activation(out=expt[:], in_=scores[:, L : L + 1], func=ACTF.Exp)
                exps = small.tile([P, L], F32, tag="exps")
                nc.scalar.activation(out=exps[:], in_=scores[:, 0:L], func=ACTF.Exp)
                expsm = small.tile([P, L], F32, tag="expsm")
                nc.vector.tensor_tensor(out=expsm[:], in0=exps[:], in1=mask[:], op=ALU.mult)
                c0 = small.tile([P, 1], F32, tag="c0")
                nc.vector.tensor_tensor(out=c0[:], in0=sumexp[:], in1=expt[:], op=ALU.subtract)

                # argbuf[:, k] = c0 - sum_{j<k} expsm[j]   (exclusive prefix)
                # scan: state = (expsm[t] - state) * (-1)  => state -= expsm[t]
                argbuf = small.tile([P, L + 1], F32, tag="argbuf")
                nc.vector.tensor_copy(out=argbuf[:, 0:1], in_=c0[:])
                nc.vector.tensor_tensor_scan(
                    out=argbuf[:, 1 : L + 1], data0=expsm[:], data1=neg1[:],
                    initial=c0[:], op0=ALU.subtract, op1=ALU.mult,
                )
                logarg = small.tile([P, L], F32, tag="logarg")
                nc.scalar.activation(out=logarg[:], in_=argbuf[:, 0:L], func=ACTF.Ln)
                terms = small.tile([P, L], F32, tag="terms")
                nc.vector.tensor_tensor(out=terms[:], in0=scoresd[:, 0:L], in1=logarg[:], op=ALU.subtract)
                termsm = small.tile([P, L], F32, tag="termsm")
                nc.vector.tensor_tensor(out=termsm[:], in0=terms[:], in1=mask[:], op=ALU.mult)
                tailsum = small.tile([P, 1], F32, tag="tailsum")
                nc.vector.tensor_reduce(
                    out=tailsum[:], in_=termsm[:], axis=mybir.AxisListType.X, op=ALU.add
                )

                # neg_like = log(sum_exp) - target_score - tail + cur_w
                lse = small.tile([P, 1], F32, tag="lse")
                nc.scalar.activation(out=lse[:], in_=sumexp[:], func=ACTF.Ln)
                r1 = small.tile([P, 1], F32, tag="r1")
                nc.vector.tensor_tensor(out=r1[:], in0=lse[:], in1=scoresd[:, L : L + 1], op=ALU.subtract)
                r2 = small.tile([P, 1], F32, tag="r2")
                nc.vector.tensor_tensor(out=r2[:], in0=r1[:], in1=tailsum[:], op=ALU.subtract)
                r3 = small.tile([P, 1], F32, tag="r3")
                nc.vector.tensor_tensor(out=r3[:], in0=r2[:], in1=curw[:], op=ALU.add)
                nc.sync.dma_start(out=res_t[r0 : r0 + P, :], in_=r3[:])
    nc.compile()
    return nc


def kernel(output, target, longtail, loss_weight):
    global LAST_RESULTS, _NC_CACHE
    output = np.ascontiguousarray(np.asarray(output, dtype=np.float32))
    tgt = np.ascontiguousarray(np.asarray(target).astype(np.int32).reshape(B, 1))
    lt = np.ascontiguousarray(np.asarray(longtail).astype(np.int32))
    lw = np.ascontiguousarray(np.asarray(loss_weight, dtype=np.float32))

    if _NC_CACHE is None:
        _NC_CACHE = build_nc()
    nc = _NC_CACHE

    in_maps = []
    for c in range(NCORES):
        s = slice(c * RPC, (c + 1) * RPC)
        in_maps.append(
            {"output": output[s], "tgt": tgt[s], "lt": lt[s], "lw": lw.reshape(1, V)}
        )
    LAST_RESULTS = run_bass_kernel_spmd(
        nc, in_maps, core_ids=list(range(NCORES)), trace=TRACE, **TRACE_KWARGS
    )
    return np.concatenate(
        [r["neg_like"].reshape(-1) for r in LAST_RESULTS.results], axis=0
    ).astype(np.float32)



# revision 8
# speedup vs baseline: 1.1725x; 1.1725x over previous
"""Trainium2 Bass kernel for the longtail Plackett-Luce loss.

Math (per batch row b):
    sum_exp  = sum_v exp(output[b, v])
    log_pl   = output[b, target[b]] - log(sum_exp)
    exp_s[k] = mask[k] * exp(output[b, longtail[b, k]])     mask = longtail > 0
    arg[k]   = (sum_exp - exp(output[b, target[b]])) - sum_{j<k} exp_s[j]
    tail     = sum_k mask[k] * (scores[k] - log(arg[k]))
    neg_like = -(log_pl + tail) + loss_weight[target[b]]

Sharding: batch rows split across 8 NeuronCores (512 rows each), loss_weight
replicated.

Per core the 512x32000 f32 slice is streamed through SBUF in half-row tiles
X[128, 16001] (two buffers double-buffer the halves); the scalar engine does
exp in place with a fused row-sum (accum_out).  Column 16000 of each half is
a 0.0 sentinel.

The 51 per-row gathers (50 longtail + target) use gpsimd ap_gather: each
16-partition core gathers its rows' union list (52 slots x 16 rows = 832
shared positions) from the exp'd half in ~400 ns.  Row p's own values land
contiguously at union columns i with i%16 == p%16 (slot s = i//16; s=0 is
the target, s>=1 the tail list).  Indices >= 16000 are redirected to the
sentinel in half 0 and rebased in half 1, so merged = gatherA + gatherB.
Pad slots (longtail <= 0) point at the sentinel in both halves -> merged 0.

The whole loss then stays in union layout: with eye[p,i] = (i%16 == p%16),
scan_data = merged*eye feeds one exclusive suffix-mass scan seeded with
sum_exp; the target slot sits first in each row block, so the scan state at
tail slot k is exactly arg[k], and the target slot's own term
(ln(merged_t) - ln(sum_exp)) = log_pl.  One masked reduce over the 832
columns yields log_pl + tail, and neg_like = cur_w - reduce.
"""

import sys

import numpy as np

sys.path.insert(0, "/opt/trn_rl_repo")

import concourse.bass as bass  # noqa: E402
import concourse.bacc as bacc  # noqa: E402
import concourse.tile as tile  # noqa: E402
from concourse import mybir  # noqa: E402
from concourse.bass_utils import run_bass_kernel_spmd  # noqa: E402

B, V, L = 4096, 32000, 50
NCORES = 8
RPC = B // NCORES   # 512 rows per core
P = 128             # SBUF partitions
G = RPC // P        # 4 row-groups per core
S = L + 2           # 52 slots per row: target + 50 tail (s=0 target)
NI = S * 16         # 832 union positions per 16-partition core
H = V // 2          # 16000: half width
CH = H // 2         # 8000: DMA/exp chunk

F32 = mybir.dt.float32
I32 = mybir.dt.int32
I16 = mybir.dt.int16
ALU = mybir.AluOpType
ACTF = mybir.ActivationFunctionType

# Knobs test.py can flip for profiling.
TRACE = False
TRACE_KWARGS = {}
LAST_RESULTS = None
DEBUG = False

_NC_CACHE = None


def build_nc():
    nc = bacc.Bacc()
    out_t = nc.dram_tensor("output", [RPC, V], F32, kind="ExternalInput")
    w_t = nc.dram_tensor("wcat", [RPC, S], I16, kind="ExternalInput")
    tgt_t = nc.dram_tensor("tgt", [RPC, 1], I32, kind="ExternalInput")
    lw_t = nc.dram_tensor("lw", [1, V], F32, kind="ExternalInput")
    res_t = nc.dram_tensor("neg_like", [RPC, 1], F32, kind="ExternalOutput")
    if DEBUG:
        dbg_merged_t = nc.dram_tensor("dbg_merged", [RPC, NI], F32, kind="ExternalOutput")
        dbg_sumexp_t = nc.dram_tensor("dbg_sumexp", [RPC, 1], F32, kind="ExternalOutput")
        dbg_eye_t = nc.dram_tensor("dbg_eye", [P, NI], F32, kind="ExternalOutput")

    out_ap = out_t[:, :]

    with tile.TileContext(nc) as tc:
        with (
            tc.tile_pool(name="xpool", bufs=2) as xpool,
            tc.tile_pool(name="upool", bufs=2) as upool,
            tc.tile_pool(name="spool", bufs=2) as spool,
            tc.tile_pool(name="consts", bufs=1) as consts,
        ):
            # ---- one-time constants ----
            # eye[p, i] = 1.0 iff i % 16 == p % 16  (own-block mask in union layout)
            eyei = consts.tile([P, NI], I32)
            nc.gpsimd.iota(eyei[:], pattern=[[0, S], [1, 16]], base=128,
                           channel_multiplier=-1)
            eyea = consts.tile([P, NI], I32)
            nc.vector.tensor_scalar(out=eyea[:], in0=eyei[:], scalar1=15,
                                    scalar2=None, op0=ALU.bitwise_and)
            eye = consts.tile([P, NI], F32)
            nc.vector.tensor_scalar(out=eye[:], in0=eyea[:], scalar1=0,
                                    scalar2=None, op0=ALU.is_equal)
            if DEBUG:
                nc.sync.dma_start(out=dbg_eye_t[:, :], in_=eye[:])
            neg1 = consts.tile([P, NI], F32)
            nc.vector.memset(neg1[:], -1.0)

            for g in range(G):
                r0 = g * P

                # ---- index prep (slot layout [P, S], int16) ----
                w_sb = spool.tile([P, S], I16, tag="w")
                nc.sync.dma_start(out=w_sb[:], in_=w_t[r0 : r0 + P, :])
                tgt_sb = spool.tile([P, 1], I32, tag="tgt")
                nc.sync.dma_start(out=tgt_sb[:], in_=tgt_t[r0 : r0 + P, :])

                # half 0: idxA = min(w, 16000); pad slots (w==0, s>=1) -> 16000
                idxA = spool.tile([P, S], I16, tag="idxA")
                nc.vector.tensor_scalar(out=idxA[:], in0=w_sb[:], scalar1=H,
                                        scalar2=None, op0=ALU.min)
                padk = spool.tile([P, S], I16, tag="padk")
                nc.vector.tensor_scalar(out=padk[:], in0=w_sb[:], scalar1=0,
                                        scalar2=H, op0=ALU.is_equal, op1=ALU.mult)
                nc.vector.tensor_tensor(out=idxA[:, 1:S], in0=idxA[:, 1:S],
                                        in1=padk[:, 1:S], op=ALU.add)
                # half 1: idxB = w - 16000 if w >= 16000 else 16000
                selB = spool.tile([P, S], I16, tag="selB")
                nc.vector.tensor_scalar(out=selB[:], in0=w_sb[:], scalar1=H,
                                        scalar2=None, op0=ALU.is_ge)
                idxB = spool.tile([P, S], I16, tag="idxB")
                nc.vector.tensor_tensor(out=idxB[:], in0=w_sb[:], in1=selB[:],
                                        op=ALU.mult)
                selC = spool.tile([P, S], I16, tag="selC")
                nc.vector.tensor_scalar(out=selC[:], in0=selB[:], scalar1=-2 * H,
                                        scalar2=H, op0=ALU.mult, op1=ALU.add)
                nc.vector.tensor_tensor(out=idxB[:], in0=idxB[:], in1=selC[:],
                                        op=ALU.add)

                # loss_weight[target]
                curw = spool.tile([P, 1], F32, tag="curw")
                nc.gpsimd.indirect_dma_start(
                    out=curw[:], out_offset=None,
                    in_=lw_t[:, :],
                    in_offset=bass.IndirectOffsetOnAxis(ap=tgt_sb[:], axis=1),
                )

                # ---- stream both halves; exp in place; gather each half ----
                acc = spool.tile([P, 4], F32, tag="acc")
                gAB = []
                for h in range(2):
                    x = xpool.tile([P, H + 1], F32, tag="x")
                    nc.vector.memset(x[:, H : H + 1], 0.0)
                    for c in range(2):
                        nc.sync.dma_start(
                            out=x[:, c * CH : (c + 1) * CH],
                            in_=out_ap[r0 : r0 + P,
                                       h * H + c * CH : h * H + (c + 1) * CH],
                        )
                        nc.scalar.activation(
                            out=x[:, c * CH : (c + 1) * CH],
                            in_=x[:, c * CH : (c + 1) * CH],
                            func=ACTF.Exp,
                            accum_out=acc[:, 2 * h + c : 2 * h + c + 1],
                        )
                    gh = upool.tile([P, NI], F32, tag=f"g{h}")
                    nc.gpsimd.ap_gather(
                        out_ap=gh[:], in_ap=x[:],
                        idxs_ap=(idxA if h == 0 else idxB)[:],
                        channels=P, num_elems=H + 1, d=1, num_idxs=NI,
                    )
                    gAB.append(gh)

                sumexp = spool.tile([P, 1], F32, tag="sumexp")
                nc.vector.tensor_reduce(out=sumexp[:], in_=acc[:],
                                        axis=mybir.AxisListType.X, op=ALU.add)

                # ---- union-layout tail math ----
                gA, gB = gAB
                # merged exp'd values (0 at pad slots and half-misses)
                nc.vector.tensor_tensor(out=gA[:], in0=gA[:], in1=gB[:], op=ALU.add)
                merged = gA
                if DEBUG:
                    nc.sync.dma_start(out=dbg_merged_t[r0 : r0 + P, :], in_=merged[:])
                    nc.sync.dma_start(out=dbg_sumexp_t[r0 : r0 + P, :], in_=sumexp[:])
                padm = upool.tile([P, NI], F32, tag="padm")
                nc.vector.tensor_scalar(out=padm[:], in0=merged[:], scalar1=0,
                                        scalar2=None, op0=ALU.is_gt)
                # safe = merged + (1 - padm): pads -> 1.0 so Ln stays finite
                nc.vector.tensor_scalar(out=gB[:], in0=padm[:], scalar1=-1.0,
                                        scalar2=1.0, op0=ALU.mult, op1=ALU.add)
                nc.vector.tensor_tensor(out=gB[:], in0=merged[:], in1=gB[:], op=ALU.add)
                safe = gB

                # scan_data = merged * eye (own-block contributions only)
                sdat = upool.tile([P, NI], F32, tag="sdat")
                nc.vector.tensor_tensor(out=sdat[:], in0=merged[:], in1=eye[:], op=ALU.mult)

                # argbuf[:, i] = sum_exp - sum_{j<i} sdat[j]  (exclusive)
                argbuf = upool.tile([P, NI + 1], F32, tag="argbuf")
                nc.vector.tensor_copy(out=argbuf[:, 0:1], in_=sumexp[:])
                nc.vector.tensor_tensor_scan(
                    out=argbuf[:, 1 : NI + 1], data0=sdat[:], data1=neg1[:],
                    initial=sumexp[:], op0=ALU.subtract, op1=ALU.mult,
                )

                # scores_raw = Ln(safe); ln_arg = Ln(argbuf[:, :NI])
                nc.scalar.activation(out=safe[:], in_=safe[:], func=ACTF.Ln)
                nc.scalar.activation(out=sdat[:], in_=argbuf[:, 0:NI], func=ACTF.Ln)

                # contrib = (scores_raw - ln_arg) * padm * eye; sum over columns
                nc.vector.tensor_tensor(out=safe[:], in0=safe[:], in1=sdat[:], op=ALU.subtract)
                nc.vector.tensor_tensor(out=safe[:], in0=safe[:], in1=padm[:], op=ALU.mult)
                nc.vector.tensor_tensor(out=safe[:], in0=safe[:], in1=eye[:], op=ALU.mult)
                total = spool.tile([P, 1], F32, tag="total")
                nc.vector.tensor_reduce(out=total[:], in_=safe[:],
                                        axis=mybir.AxisListType.X, op=ALU.add)

                # neg_like = cur_w - (log_pl + tail)
                res = spool.tile([P, 1], F32, tag="res")
                nc.vector.tensor_tensor(out=res[:], in0=curw[:], in1=total[:], op=ALU.subtract)
                nc.sync.dma_start(out=res_t[r0 : r0 + P, :], in_=res[:])
    nc.compile()
    return nc


def kernel(output, target, longtail, loss_weight):
    global LAST_RESULTS, _NC_CACHE
    output = np.ascontiguousarray(np.asarray(output, dtype=np.float32))
    tgt64 = np.asarray(target).astype(np.int64).reshape(B, 1)
    lt64 = np.asarray(longtail).astype(np.int64)
    lw = np.ascontiguousarray(np.asarray(loss_weight, dtype=np.float32))

    # slot layout: col 0 = target, cols 1..50 = clipped longtail, col 51 pad(0)
    wcat = np.zeros((B, S), dtype=np.int16)
    wcat[:, 0] = np.clip(tgt64[:, 0], 0, V - 1).astype(np.int16)
    wcat[:, 1 : L + 1] = np.clip(lt64, 0, V - 1).astype(np.int16)
    tgt = np.ascontiguousarray(tgt64.astype(np.int32))

    if _NC_CACHE is None:
        _NC_CACHE = build_nc()
    nc = _NC_CACHE

    in_maps = []
    for c in range(NCORES):
        s = slice(c * RPC, (c + 1) * RPC)
        in_maps.append(
            {"output": output[s], "wcat": wcat[s], "tgt": tgt[s],
             "lw": lw.reshape(1, V)}
        )
    LAST_RESULTS = run_bass_kernel_spmd(
        nc, in_maps, core_ids=list(range(NCORES)), trace=TRACE, **TRACE_KWARGS
    )
    return np.concatenate(
        [r["neg_like"].reshape(-1) for r in LAST_RESULTS.results], axis=0
    ).astype(np.float32)


# revision 10
# speedup vs baseline: 1.1963x; 1.0203x over previous
"""Trainium2 Bass kernel for the longtail Plackett-Luce loss.

Math (per batch row b):
    sum_exp  = sum_v exp(output[b, v])
    log_pl   = output[b, target[b]] - log(sum_exp)
    exp_s[k] = mask[k] * exp(output[b, longtail[b, k]])     mask = longtail > 0
    arg[k]   = (sum_exp - exp(output[b, target[b]])) - sum_{j<k} exp_s[j]
    tail     = sum_k mask[k] * (scores[k] - log(arg[k]))
    neg_like = -(log_pl + tail) + loss_weight[target[b]]

Sharding: batch rows split across 8 NeuronCores (512 rows each), loss_weight
replicated.

Per core the 512x32000 f32 slice is streamed through SBUF in half-row tiles
X[128, 16001] (two buffers double-buffer the halves); the scalar engine does
exp in place with a fused row-sum (accum_out).  Column 16000 of each half is
a 0.0 sentinel.

The 51 per-row gathers (50 longtail + target) use gpsimd ap_gather: each
16-partition core gathers its rows' union list (52 slots x 16 rows = 832
shared positions) from the exp'd half in ~400 ns.  Row p's own values land
contiguously at union columns i with i%16 == p%16 (slot s = i//16; s=0 is
the target, s>=1 the tail list).  Indices >= 16000 are redirected to the
sentinel in half 0 and rebased in half 1, so merged = gatherA + gatherB.
Pad slots (longtail <= 0) point at the sentinel in both halves -> merged 0.

The whole loss then stays in union layout: with eye[p,i] = (i%16 == p%16),
scan_data = merged*eye feeds one exclusive suffix-mass scan seeded with
sum_exp; the target slot sits first in each row block, so the scan state at
tail slot k is exactly arg[k], and the target slot's own term
(ln(merged_t) - ln(sum_exp)) = log_pl.  One masked reduce over the 832
columns yields log_pl + tail, and neg_like = cur_w - reduce.
"""

import sys

import numpy as np

sys.path.insert(0, "/opt/trn_rl_repo")

import concourse.bass as bass  # noqa: E402
import concourse.bacc as bacc  # noqa: E402
import concourse.tile as tile  # noqa: E402
from concourse import mybir  # noqa: E402
from concourse.bass_utils import run_bass_kernel_spmd  # noqa: E402

B, V, L = 4096, 32000, 50
NCORES = 8
RPC = B // NCORES   # 512 rows per core
P = 128             # SBUF partitions
G = RPC // P        # 4 row-groups per core
S = L + 2           # 52 slots per row: target + 50 tail (s=0 target)
NI = S * 16         # 832 union positions per 16-partition core
H = V // 2          # 16000: half width
CH = H // 2         # 8000: DMA/exp chunk

F32 = mybir.dt.float32
I32 = mybir.dt.int32
I16 = mybir.dt.int16
ALU = mybir.AluOpType
ACTF = mybir.ActivationFunctionType

# Knobs test.py can flip for profiling.
TRACE = False
TRACE_KWARGS = {}
LAST_RESULTS = None
DEBUG = False

_NC_CACHE = None


def _pin_act_table(nc):
    """Make every ACT func set except the combined exp+ln one claim no
    functions, so the table-load pass picks natural_log_exp_and_others for
    both Exp and Ln -> exactly one ACT_TABLE_LOAD instead of per-group
    ping-pong.  Set ids are positional, and we only blank other sets'
    claimed contents, so the emitted id still names the right table."""
    from concourse.hw_specs import get_activation_tables

    tables = get_activation_tables(nc.m.arch)
    assert "natural_log_exp_and_others" in tables
    for name, funcs in tables.items():
        if name != "natural_log_exp_and_others":
            funcs.clear()


def build_nc():
    nc = bacc.Bacc()
    _pin_act_table(nc)
    out_t = nc.dram_tensor("output", [RPC, V], F32, kind="ExternalInput")
    w_t = nc.dram_tensor("wcat", [RPC, S], I16, kind="ExternalInput")
    tgt_t = nc.dram_tensor("tgt", [RPC, 1], I32, kind="ExternalInput")
    lw_t = nc.dram_tensor("lw", [1, V], F32, kind="ExternalInput")
    res_t = nc.dram_tensor("neg_like", [RPC, 1], F32, kind="ExternalOutput")
    if DEBUG:
        dbg_merged_t = nc.dram_tensor("dbg_merged", [RPC, NI], F32, kind="ExternalOutput")
        dbg_sumexp_t = nc.dram_tensor("dbg_sumexp", [RPC, 1], F32, kind="ExternalOutput")
        dbg_eye_t = nc.dram_tensor("dbg_eye", [P, NI], F32, kind="ExternalOutput")

    out_ap = out_t[:, :]

    with tile.TileContext(nc) as tc:
        with (
            tc.tile_pool(name="xpool", bufs=2) as xpool,
            tc.tile_pool(name="upool", bufs=2) as upool,
            tc.tile_pool(name="spool", bufs=2) as spool,
            tc.tile_pool(name="consts", bufs=1) as consts,
        ):
            # ---- one-time constants ----
            # eye[p, i] = 1.0 iff i % 16 == p % 16  (own-block mask in union layout)
            eyei = consts.tile([P, NI], I32)
            nc.gpsimd.iota(eyei[:], pattern=[[0, S], [1, 16]], base=128,
                           channel_multiplier=-1)
            eyea = consts.tile([P, NI], I32)
            nc.vector.tensor_scalar(out=eyea[:], in0=eyei[:], scalar1=15,
                                    scalar2=None, op0=ALU.bitwise_and)
            eye = consts.tile([P, NI], F32)
            nc.vector.tensor_scalar(out=eye[:], in0=eyea[:], scalar1=0,
                                    scalar2=None, op0=ALU.is_equal)
            if DEBUG:
                nc.sync.dma_start(out=dbg_eye_t[:, :], in_=eye[:])
            neg1 = consts.tile([P, NI], F32)
            nc.vector.memset(neg1[:], -1.0)

            def emit_stream(g):
                """Index prep + stream both halves (exp in place) + gathers."""
                r0 = g * P
                st = {}

                w_sb = spool.tile([P, S], I16, tag="w", name=f"w{g}")
                nc.sync.dma_start(out=w_sb[:], in_=w_t[r0 : r0 + P, :])
                tgt_sb = spool.tile([P, 1], I32, tag="tgt", name=f"tgt{g}")
                nc.sync.dma_start(out=tgt_sb[:], in_=tgt_t[r0 : r0 + P, :])

                # half 0: idxA = min(w, 16000); pad slots (w==0, s>=1) -> 16000
                idxA = spool.tile([P, S], I16, tag="idxA", name=f"idxA{g}")
                nc.vector.tensor_scalar(out=idxA[:], in0=w_sb[:], scalar1=H,
                                        scalar2=None, op0=ALU.min)
                padk = spool.tile([P, S], I16, tag="padk", name=f"padk{g}")
                nc.vector.tensor_scalar(out=padk[:], in0=w_sb[:], scalar1=0,
                                        scalar2=H, op0=ALU.is_equal, op1=ALU.mult)
                nc.vector.tensor_tensor(out=idxA[:, 1:S], in0=idxA[:, 1:S],
                                        in1=padk[:, 1:S], op=ALU.add)
                # half 1: idxB = w - 16000 if w >= 16000 else 16000
                selB = spool.tile([P, S], I16, tag="selB", name=f"selB{g}")
                nc.vector.tensor_scalar(out=selB[:], in0=w_sb[:], scalar1=H,
                                        scalar2=None, op0=ALU.is_ge)
                idxB = spool.tile([P, S], I16, tag="idxB", name=f"idxB{g}")
                nc.vector.tensor_tensor(out=idxB[:], in0=w_sb[:], in1=selB[:],
                                        op=ALU.mult)
                selC = spool.tile([P, S], I16, tag="selC", name=f"selC{g}")
                nc.vector.tensor_scalar(out=selC[:], in0=selB[:], scalar1=-2 * H,
                                        scalar2=H, op0=ALU.mult, op1=ALU.add)
                nc.vector.tensor_tensor(out=idxB[:], in0=idxB[:], in1=selC[:],
                                        op=ALU.add)

                # loss_weight[target]
                curw = spool.tile([P, 1], F32, tag="curw", name=f"curw{g}")
                nc.gpsimd.indirect_dma_start(
                    out=curw[:], out_offset=None,
                    in_=lw_t[:, :],
                    in_offset=bass.IndirectOffsetOnAxis(ap=tgt_sb[:], axis=1),
                )
                st["curw"] = curw

                # stream both halves; exp in place; gather each half
                acc = spool.tile([P, 4], F32, tag="acc", name=f"acc{g}")
                gAB = []
                for h in range(2):
                    x = xpool.tile([P, H + 1], F32, tag="x", name=f"x{g}_{h}")
                    nc.vector.memset(x[:, H : H + 1], 0.0)
                    for c in range(2):
                        nc.sync.dma_start(
                            out=x[:, c * CH : (c + 1) * CH],
                            in_=out_ap[r0 : r0 + P,
                                       h * H + c * CH : h * H + (c + 1) * CH],
                        )
                        nc.scalar.activation(
                            out=x[:, c * CH : (c + 1) * CH],
                            in_=x[:, c * CH : (c + 1) * CH],
                            func=ACTF.Exp,
                            accum_out=acc[:, 2 * h + c : 2 * h + c + 1],
                        )
                    gh = upool.tile([P, NI], F32, tag=f"g{h}", name=f"g{h}_{g}")
                    nc.gpsimd.ap_gather(
                        out_ap=gh[:], in_ap=x[:],
                        idxs_ap=(idxA if h == 0 else idxB)[:],
                        channels=P, num_elems=H + 1, d=1, num_idxs=NI,
                    )
                    gAB.append(gh)
                st["gAB"] = gAB

                sumexp = spool.tile([P, 1], F32, tag="sumexp", name=f"sumexp{g}")
                nc.vector.tensor_reduce(out=sumexp[:], in_=acc[:],
                                        axis=mybir.AxisListType.X, op=ALU.add)
                st["sumexp"] = sumexp
                return st

            def emit_tail(g, st):
                """Union-layout tail math + result write for group g."""
                r0 = g * P
                gA, gB = st["gAB"]
                sumexp, curw = st["sumexp"], st["curw"]

                # merged exp'd values (0 at pad slots and half-misses)
                nc.vector.tensor_tensor(out=gA[:], in0=gA[:], in1=gB[:], op=ALU.add)
                merged = gA
                if DEBUG:
                    nc.sync.dma_start(out=dbg_merged_t[r0 : r0 + P, :], in_=merged[:])
                    nc.sync.dma_start(out=dbg_sumexp_t[r0 : r0 + P, :], in_=sumexp[:])
                padm = upool.tile([P, NI], F32, tag="padm", name=f"padm{g}")
                nc.vector.tensor_scalar(out=padm[:], in0=merged[:], scalar1=0,
                                        scalar2=None, op0=ALU.is_gt)
                # safe = merged + (1 - padm): pads -> 1.0 so Ln stays finite
                nc.vector.tensor_scalar(out=gB[:], in0=padm[:], scalar1=-1.0,
                                        scalar2=1.0, op0=ALU.mult, op1=ALU.add)
                nc.vector.tensor_tensor(out=gB[:], in0=merged[:], in1=gB[:], op=ALU.add)
                safe = gB

                # scan_data = merged * eye (own-block contributions only)
                sdat = upool.tile([P, NI], F32, tag="sdat", name=f"sdat{g}")
                nc.vector.tensor_tensor(out=sdat[:], in0=merged[:], in1=eye[:], op=ALU.mult)

                # argbuf[:, i] = sum_exp - sum_{j<i} sdat[j]  (exclusive)
                argbuf = upool.tile([P, NI + 1], F32, tag="argbuf", name=f"argbuf{g}")
                nc.vector.tensor_copy(out=argbuf[:, 0:1], in_=sumexp[:])
                nc.vector.tensor_tensor_scan(
                    out=argbuf[:, 1 : NI + 1], data0=sdat[:], data1=neg1[:],
                    initial=sumexp[:], op0=ALU.subtract, op1=ALU.mult,
                )

                # scores_raw = Ln(safe); ln_arg = Ln(argbuf[:, :NI])
                nc.scalar.activation(out=safe[:], in_=safe[:], func=ACTF.Ln)
                nc.scalar.activation(out=sdat[:], in_=argbuf[:, 0:NI], func=ACTF.Ln)

                # contrib = (scores_raw - ln_arg) * padm * eye; sum over columns
                nc.vector.tensor_tensor(out=safe[:], in0=safe[:], in1=sdat[:], op=ALU.subtract)
                nc.vector.tensor_tensor(out=safe[:], in0=safe[:], in1=padm[:], op=ALU.mult)
                nc.vector.tensor_tensor(out=safe[:], in0=safe[:], in1=eye[:], op=ALU.mult)
                total = spool.tile([P, 1], F32, tag="total", name=f"total{g}")
                nc.vector.tensor_reduce(out=total[:], in_=safe[:],
                                        axis=mybir.AxisListType.X, op=ALU.add)

                # neg_like = cur_w - (log_pl + tail)
                res = spool.tile([P, 1], F32, tag="res", name=f"res{g}")
                nc.vector.tensor_tensor(out=res[:], in0=curw[:], in1=total[:], op=ALU.subtract)
                nc.sync.dma_start(out=res_t[r0 : r0 + P, :], in_=res[:])

            # software-pipelined emission: group g's tail is emitted after
            # group g+1's stream, so no engine queue head-blocks the stream
            st_prev = None
            for g in range(G):
                st_cur = emit_stream(g)
                if st_prev is not None:
                    emit_tail(g - 1, st_prev)
                st_prev = st_cur
            emit_tail(G - 1, st_prev)
    nc.compile()
    return nc


def kernel(output, target, longtail, loss_weight):
    global LAST_RESULTS, _NC_CACHE
    output = np.ascontiguousarray(np.asarray(output, dtype=np.float32))
    tgt64 = np.asarray(target).astype(np.int64).reshape(B, 1)
    lt64 = np.asarray(longtail).astype(np.int64)
    lw = np.ascontiguousarray(np.asarray(loss_weight, dtype=np.float32))

    # slot layout: col 0 = target, cols 1..50 = clipped longtail, col 51 pad(0)
    wcat = np.zeros((B, S), dtype=np.int16)
    wcat[:, 0] = np.clip(tgt64[:, 0], 0, V - 1).astype(np.int16)
    wcat[:, 1 : L + 1] = np.clip(lt64, 0, V - 1).astype(np.int16)
    tgt = np.ascontiguousarray(tgt64.astype(np.int32))

    if _NC_CACHE is None:
        _NC_CACHE = build_nc()
    nc = _NC_CACHE

    in_maps = []
    for c in range(NCORES):
        s = slice(c * RPC, (c + 1) * RPC)
        in_maps.append(
            {"output": output[s], "wcat": wcat[s], "tgt": tgt[s],
             "lw": lw.reshape(1, V)}
        )
    LAST_RESULTS = run_bass_kernel_spmd(
        nc, in_maps, core_ids=list(range(NCORES)), trace=TRACE, **TRACE_KWARGS
    )
    return np.concatenate(
        [r["neg_like"].reshape(-1) for r in LAST_RESULTS.results], axis=0
    ).astype(np.float32)
